# revision 15
# baseline (speedup 1.0000x reference)
"""Triplane embedding-lookup + MLP kernel for Trainium2 (8 NeuronCores).

Strategy:
  - Host: build a "patch table" PT[3*512*512, 128] where row (pl,y,x) holds the
    4 bilinear-corner pixel vectors [p(y,x), p(y,x+1), p(y+1,x), p(y+1,x+1)]
    (32 channels each). One indirect-DMA descriptor then fetches all data a
    point needs from one plane.
  - Shard the N=1M points across 8 cores (data parallel, planes replicated).
  - Device, per block of 128*K points: compute integer cell ids + bilinear
    weights on DVE/Pool/ACT, one indirect DMA gather (idx [128, 3K]) from PT,
    weighted-sum combine to feats[128, K*32], PE transpose to [32, pts],
    4-layer MLP on PE (bf16), result [1, pts] DMA'd to DRAM.
"""

import sys

sys.path.insert(0, "/opt/trn_rl_repo")

from contextlib import ExitStack

import numpy as np

RES = 512
CELLS = RES * RES
EMB = 32
HID = 128
N = 1_000_000
NCORES = 8

K = 32          # points per partition per block
KT = 992        # points per partition per core (31 blocks of K)
NBLK = KT // K
NP = 128 * KT   # 126976 points per core
BATCH = 4       # k-groups per MLP batch -> 512 points per matmul
NBATCH = K // BATCH

# plane -> (x_coord_index, y_coord_index); x indexes W, y indexes H
PAIRS = ((0, 1), (1, 2), (0, 2))

TABLE_F32 = True   # patch table + combine precision
LAST_RESULTS = None  # BassKernelResults of the most recent run (for test harness)

_BUILT = {}


def _build_nc(table_dt_name: str, kt: int = KT, do_finalize: bool = True):
    from concourse import bacc, bass, mybir
    import concourse.tile as tile
    from concourse.masks import make_identity

    dt = mybir.dt
    tdt = getattr(dt, table_dt_name)
    f32 = dt.float32
    i32 = dt.int32
    bf16 = dt.bfloat16
    mult = mybir.AluOpType.mult
    add = mybir.AluOpType.add
    AF = mybir.ActivationFunctionType

    nc = bacc.Bacc("TRN2", target_bir_lowering=False)

    ptd = nc.dram_tensor("pt", [3 * CELLS, 128], tdt, kind="ExternalInput")
    crd = nc.dram_tensor("coords", [128 * kt, 3], f32, kind="ExternalInput")
    w0d = nc.dram_tensor("w0t", [EMB, HID], bf16, kind="ExternalInput")
    w1d = nc.dram_tensor("w1t", [HID, HID], bf16, kind="ExternalInput")
    w2d = nc.dram_tensor("w2t", [HID, HID], bf16, kind="ExternalInput")
    w3d = nc.dram_tensor("w3t", [HID, 1], bf16, kind="ExternalInput")
    b0d = nc.dram_tensor("b0c", [HID, 1], f32, kind="ExternalInput")
    b1d = nc.dram_tensor("b1c", [HID, 1], f32, kind="ExternalInput")
    b2d = nc.dram_tensor("b2c", [HID, 1], f32, kind="ExternalInput")
    b3d = nc.dram_tensor("b3c", [1, 1], f32, kind="ExternalInput")
    outd = nc.dram_tensor("out", [kt * 128], f32, kind="ExternalOutput")

    crd3 = crd[:].rearrange("(p kt) c -> p (kt c)", p=128)
    outv = outd[:].unsqueeze(0)

    with tile.TileContext(nc) as tc, ExitStack() as ctx:
        cpool = ctx.enter_context(tc.tile_pool(name="consts", bufs=1))

        def const_tile(shape, dtp, tag):
            return cpool.tile(shape, dtp, tag=tag, name=tag)

        w0s = const_tile([EMB, HID], bf16, "w0s")
        w1s = const_tile([HID, HID], bf16, "w1s")
        w2s = const_tile([HID, HID], bf16, "w2s")
        w3s = const_tile([HID, 1], bf16, "w3s")
        b0s = const_tile([HID, 1], f32, "b0s")
        b1s = const_tile([HID, 1], f32, "b1s")
        b2s = const_tile([HID, 1], f32, "b2s")
        b3s = const_tile([1, 1], f32, "b3s")
        ident = const_tile([128, 128], bf16, "ident")
        for s, d in ((w0s, w0d), (w1s, w1d), (w2s, w2d), (w3s, w3d),
                     (b0s, b0d), (b1s, b1d), (b2s, b2d), (b3s, b3d)):
            nc.sync.dma_start(s[:], d[:])
        make_identity(nc, ident[:])

        work = ctx.enter_context(tc.tile_pool(name="work", bufs=2))
        gpool = ctx.enter_context(tc.tile_pool(name="gather", bufs=2))
        psum = ctx.enter_context(tc.tile_pool(name="psum", bufs=2, space="PSUM"))

        def wt(shape, dtp, tag, bufs=2):
            return work.tile(shape, dtp, tag=tag, name=tag, bufs=bufs)

        for b in range(kt // K):
            c = wt([128, K * 3], f32, "c")
            nc.sync.dma_start(c[:], crd3[:, b * K * 3:(b + 1) * K * 3])

            pix = wt([128, K * 3], f32, "pix")
            nc.scalar.activation(pix[:], c[:], AF.Copy, bias=255.5, scale=255.5)
            # HW f32->i32 cast is rint; cast(pix - 0.5) == floor(pix) for
            # non-integer pix (integer pix may give pix-1 with fr=1.0, which is
            # bilinear-equivalent).
            pixm = wt([128, K * 3], f32, "pixm")
            nc.scalar.activation(pixm[:], c[:], AF.Copy, bias=255.0, scale=255.5)
            ci = wt([128, K * 3], i32, "ci")
            nc.gpsimd.tensor_copy(ci[:], pixm[:])
            cf = wt([128, K * 3], f32, "cf")
            nc.gpsimd.tensor_copy(cf[:], ci[:])
            fr = wt([128, K * 3], f32, "fr")
            nc.vector.tensor_sub(fr[:], pix[:], cf[:])
            omf = wt([128, K * 3], f32, "omf")
            nc.scalar.activation(omf[:], fr[:], AF.Copy, bias=1.0, scale=-1.0)

            ci3 = ci[:].rearrange("p (k c) -> p k c", c=3)
            fr3 = fr[:].rearrange("p (k c) -> p k c", c=3)
            omf3 = omf[:].rearrange("p (k c) -> p k c", c=3)

            idx = wt([128, 3 * K], i32, "idx")
            idx3 = idx[:].rearrange("p (pl k) -> p pl k", pl=3)
            for pl, (xc, yc) in enumerate(PAIRS):
                t1 = wt([128, K], i32, "t1", bufs=3)
                nc.vector.scalar_tensor_tensor(
                    out=t1[:], in0=ci3[:, :, yc], scalar=RES, in1=ci3[:, :, xc],
                    op0=mult, op1=add)
                if pl == 0:
                    nc.gpsimd.tensor_copy(idx3[:, 0], t1[:])
                else:
                    nc.gpsimd.tensor_scalar_add(idx3[:, pl], t1[:], pl * CELLS)

            # corner weights: [(1-fy)(1-fx), (1-fy)fx, fy(1-fx), fy fx]
            wts = wt([128, 3 * 4 * K], f32, "wts")
            wts4 = wts[:].rearrange("p (pl c k) -> p pl c k", pl=3, c=4)
            for pl, (xc, yc) in enumerate(PAIRS):
                fx, fy = fr3[:, :, xc], fr3[:, :, yc]
                gx, gy = omf3[:, :, xc], omf3[:, :, yc]
                eng = nc.vector if pl != 1 else nc.gpsimd
                eng.tensor_tensor(out=wts4[:, pl, 0], in0=gy, in1=gx, op=mult)
                eng.tensor_tensor(out=wts4[:, pl, 1], in0=gy, in1=fx, op=mult)
                eng.tensor_tensor(out=wts4[:, pl, 2], in0=fy, in1=gx, op=mult)
                eng.tensor_tensor(out=wts4[:, pl, 3], in0=fy, in1=fx, op=mult)

            # HW indirect DMA uses ONE offset per partition and fetches the
            # whole dest row contiguously, so each (plane, point) patch fetch
            # needs its own [P,1]-offset gather.
            g = gpool.tile([128, 3 * K * 128], tdt, tag="g", name="g")
            for col in range(3 * K):
                nc.gpsimd.indirect_dma_start(
                    out=g[:, col * 128:(col + 1) * 128], out_offset=None,
                    in_=ptd[:],
                    in_offset=_ioa()(ap=idx[:, col:col + 1], axis=0))
            g5 = g[:].rearrange("p (pl k c f) -> p pl k c f", pl=3, c=4, f=EMB)

            # combine on two engines: DVE takes 6 corner terms, Pool takes 6
            accA = wt([128, K * EMB], f32, "accA")
            accB = wt([128, K * EMB], f32, "accB")
            accA3 = accA[:].rearrange("p (k f) -> p k f", f=EMB)
            accB3 = accB[:].rearrange("p (k f) -> p k f", f=EMB)
            terms = [(pl, cc) for pl in range(3) for cc in range(4)]
            feats = wt([128, K * EMB], bf16, "feats")
            for half, eng, acc, acc3 in (
                (terms[:6], nc.vector, accA, accA3),
                (terms[6:], nc.gpsimd, accB, accB3),
            ):
                for i, (pl, cc) in enumerate(half):
                    w_b = wts4[:, pl, cc].unsqueeze(2).to_broadcast([128, K, EMB])
                    gsl = g5[:, pl, :, cc]
                    if i == 0:
                        eng.tensor_tensor(out=acc3, in0=gsl, in1=w_b, op=mult)
                    else:
                        prod = wt([128, K * EMB], f32,
                                  "prodA" if eng is nc.vector else "prodB", bufs=2)
                        eng.tensor_tensor(out=prod[:].rearrange(
                            "p (k f) -> p k f", f=EMB), in0=gsl, in1=w_b, op=mult)
                        eng.tensor_add(out=acc[:], in0=acc[:], in1=prod[:])
            nc.vector.tensor_add(out=feats[:], in0=accA[:], in1=accB[:])

            # MLP
            for j in range(NBATCH):
                k0 = b * K + j * BATCH
                ftp = psum.tile([EMB, BATCH * 128], bf16, tag="ftp", name="ftp",
                                space="PSUM", bufs=2)
                for kk in range(BATCH):
                    nc.tensor.transpose(
                        out=ftp[:, kk * 128:(kk + 1) * 128],
                        in_=feats[:, (j * BATCH + kk) * EMB:(j * BATCH + kk + 1) * EMB],
                        identity=ident[:])
                fts = wt([EMB, BATCH * 128], bf16, "fts")
                nc.scalar.activation(fts[:], ftp[:], AF.Copy)

                mm0 = psum.tile([HID, BATCH * 128], f32, tag="mm", name="mm",
                                space="PSUM", bufs=3)
                nc.tensor.matmul(out=mm0[:], lhsT=w0s[:], rhs=fts[:],
                                 start=True, stop=True)
                h0 = wt([HID, BATCH * 128], bf16, "h0")
                nc.scalar.activation(h0[:], mm0[:], AF.Relu, bias=b0s[:, 0:1])

                mm1 = psum.tile([HID, BATCH * 128], f32, tag="mm", name="mm",
                                space="PSUM", bufs=3)
                nc.tensor.matmul(out=mm1[:], lhsT=w1s[:], rhs=h0[:],
                                 start=True, stop=True)
                h1 = wt([HID, BATCH * 128], bf16, "h1")
                nc.scalar.activation(h1[:], mm1[:], AF.Relu, bias=b1s[:, 0:1])

                mm2 = psum.tile([HID, BATCH * 128], f32, tag="mm", name="mm",
                                space="PSUM", bufs=3)
                nc.tensor.matmul(out=mm2[:], lhsT=w2s[:], rhs=h1[:],
                                 start=True, stop=True)
                h2 = wt([HID, BATCH * 128], bf16, "h2")
                nc.scalar.activation(h2[:], mm2[:], AF.Relu, bias=b2s[:, 0:1])

                mm3 = psum.tile([1, BATCH * 128], f32, tag="mm3", name="mm3",
                                space="PSUM", bufs=2)
                nc.tensor.matmul(out=mm3[:], lhsT=w3s[:], rhs=h2[:],
                                 start=True, stop=True)
                res = wt([1, BATCH * 128], f32, "res")
                nc.scalar.activation(res[:], mm3[:], AF.Identity,
                                     bias=b3s[0:1, 0:1])
                nc.sync.dma_start(outv[:, k0 * 128:(k0 + BATCH) * 128], res[:])

    if do_finalize:
        nc.finalize()
    return nc


def _ioa():
    from concourse import bass
    return bass.IndirectOffsetOnAxis


def _get_nc():
    key = "float32" if TABLE_F32 else "bfloat16"
    if key not in _BUILT:
        _BUILT[key] = _build_nc(key)
    return _BUILT[key]


def _build_patch_table(planes: np.ndarray, np_dt) -> np.ndarray:
    # planes [3, 32, 512, 512] -> PT [3*512*512, 128]
    p = planes.transpose(0, 2, 3, 1)  # [3, H, W, C]
    pt = np.zeros((3, RES, RES, 4, EMB), dtype=np.float32)
    pt[:, :, :, 0] = p
    pt[:, :, :-1, 1] = p[:, :, 1:]
    pt[:, :-1, :, 2] = p[:, 1:]
    pt[:, :-1, :-1, 3] = p[:, 1:, 1:]
    return np.ascontiguousarray(pt.reshape(3 * CELLS, 4 * EMB)).astype(np_dt)


def kernel(**inputs: np.ndarray) -> np.ndarray:
    global LAST_RESULTS
    import ml_dtypes
    from concourse.bass_utils import run_bass_kernel_spmd

    coords = np.asarray(inputs["coordinates"], dtype=np.float32)
    planes = np.asarray(inputs["planes"], dtype=np.float32)
    bf = ml_dtypes.bfloat16
    np_tdt = np.float32 if TABLE_F32 else bf
    pt = _build_patch_table(planes, np_tdt)
    w0t = np.ascontiguousarray(inputs["w0"].T).astype(bf)
    w1t = np.ascontiguousarray(inputs["w1"].T).astype(bf)
    w2t = np.ascontiguousarray(inputs["w2"].T).astype(bf)
    w3t = np.ascontiguousarray(inputs["w3"].T).astype(bf)
    b0 = np.asarray(inputs["b0"], np.float32).reshape(HID, 1)
    b1 = np.asarray(inputs["b1"], np.float32).reshape(HID, 1)
    b2 = np.asarray(inputs["b2"], np.float32).reshape(HID, 1)
    b3 = np.asarray(inputs["b3"], np.float32).reshape(1, 1)

    n = coords.shape[0]
    coords_pad = np.zeros((NCORES * NP, 3), np.float32)
    coords_pad[:n] = coords

    in_maps = []
    for i in range(NCORES):
        in_maps.append({
            "pt": pt,
            "coords": np.ascontiguousarray(coords_pad[i * NP:(i + 1) * NP]),
            "w0t": w0t, "w1t": w1t, "w2t": w2t, "w3t": w3t,
            "b0c": b0, "b1c": b1, "b2c": b2, "b3c": b3,
        })

    nc = _get_nc()
    LAST_RESULTS = run_bass_kernel_spmd(nc, in_maps, list(range(NCORES)))
    pieces = []
    for i in range(NCORES):
        o = np.asarray(LAST_RESULTS.results[i]["out"], np.float32)
        pieces.append(o.reshape(KT, 128).T.ravel())  # -> point order p*KT+k
    full = np.concatenate(pieces)[:n]
    return full.reshape(1, n, 1).astype(np.float32)


# revision 18
# speedup vs baseline: 1.1228x; 1.1228x over previous
"""Triplane embedding-lookup + MLP kernel for Trainium2 (8 NeuronCores).

Strategy:
  - Host: build a "patch table" PT[3*512*512, 128] where row (pl,y,x) holds the
    4 bilinear-corner pixel vectors [p(y,x), p(y,x+1), p(y+1,x), p(y+1,x+1)]
    (32 channels each). One indirect-DMA descriptor then fetches all data a
    point needs from one plane.
  - Shard the N=1M points across 8 cores (data parallel, planes replicated).
  - Device, per block of 128*K points: compute integer cell ids + bilinear
    weights on DVE/Pool/ACT, one indirect DMA gather (idx [128, 3K]) from PT,
    weighted-sum combine to feats[128, K*32], PE transpose to [32, pts],
    4-layer MLP on PE (bf16), result [1, pts] DMA'd to DRAM.
"""

import sys

sys.path.insert(0, "/opt/trn_rl_repo")

from contextlib import ExitStack

import numpy as np

RES = 512
CELLS = RES * RES
EMB = 32
HID = 128
N = 1_000_000
NCORES = 8

K = 32          # points per partition per block
KT = 992        # points per partition per core (31 blocks of K)
NBLK = KT // K
NP = 128 * KT   # 126976 points per core
BATCH = 4       # k-groups per MLP batch -> 512 points per matmul
NBATCH = K // BATCH

# plane -> (x_coord_index, y_coord_index); x indexes W, y indexes H
PAIRS = ((0, 1), (1, 2), (0, 2))

TABLE_F32 = True   # patch table + combine precision
LAST_RESULTS = None  # BassKernelResults of the most recent run (for test harness)

_BUILT = {}


def _build_nc(table_dt_name: str, kt: int = KT, do_finalize: bool = True):
    from concourse import bacc, bass, mybir
    import concourse.tile as tile
    from concourse.masks import make_identity

    dt = mybir.dt
    tdt = getattr(dt, table_dt_name)
    f32 = dt.float32
    i32 = dt.int32
    bf16 = dt.bfloat16
    mult = mybir.AluOpType.mult
    add = mybir.AluOpType.add
    AF = mybir.ActivationFunctionType

    nc = bacc.Bacc("TRN2", target_bir_lowering=False)

    ptd = nc.dram_tensor("pt", [3 * CELLS, 128], tdt, kind="ExternalInput")
    crd = nc.dram_tensor("coords", [128 * kt, 3], f32, kind="ExternalInput")
    w0d = nc.dram_tensor("w0t", [EMB, HID], bf16, kind="ExternalInput")
    w1d = nc.dram_tensor("w1t", [HID, HID], bf16, kind="ExternalInput")
    w2d = nc.dram_tensor("w2t", [HID, HID], bf16, kind="ExternalInput")
    w3d = nc.dram_tensor("w3t", [HID, 1], bf16, kind="ExternalInput")
    b0d = nc.dram_tensor("b0c", [HID, 1], f32, kind="ExternalInput")
    b1d = nc.dram_tensor("b1c", [HID, 1], f32, kind="ExternalInput")
    b2d = nc.dram_tensor("b2c", [HID, 1], f32, kind="ExternalInput")
    b3d = nc.dram_tensor("b3c", [1, 1], f32, kind="ExternalInput")
    outd = nc.dram_tensor("out", [kt * 128], f32, kind="ExternalOutput")

    crd3 = crd[:].rearrange("(p kt) c -> p (kt c)", p=128)
    outv = outd[:].unsqueeze(0)

    with tile.TileContext(nc) as tc, ExitStack() as ctx:
        cpool = ctx.enter_context(tc.tile_pool(name="consts", bufs=1))

        def const_tile(shape, dtp, tag):
            return cpool.tile(shape, dtp, tag=tag, name=tag)

        w0s = const_tile([EMB, HID], bf16, "w0s")
        w1s = const_tile([HID, HID], bf16, "w1s")
        w2s = const_tile([HID, HID], bf16, "w2s")
        w3s = const_tile([HID, 1], bf16, "w3s")
        b0s = const_tile([HID, 1], f32, "b0s")
        b1s = const_tile([HID, 1], f32, "b1s")
        b2s = const_tile([HID, 1], f32, "b2s")
        b3s = const_tile([1, 1], f32, "b3s")
        ident = const_tile([128, 128], bf16, "ident")
        for s, d in ((w0s, w0d), (w1s, w1d), (w2s, w2d), (w3s, w3d),
                     (b0s, b0d), (b1s, b1d), (b2s, b2d), (b3s, b3d)):
            nc.sync.dma_start(s[:], d[:])
        make_identity(nc, ident[:])

        work = ctx.enter_context(tc.tile_pool(name="work", bufs=2))
        gpool = ctx.enter_context(tc.tile_pool(name="gather", bufs=2))
        psum = ctx.enter_context(tc.tile_pool(name="psum", bufs=2, space="PSUM"))

        def wt(shape, dtp, tag, bufs=2):
            return work.tile(shape, dtp, tag=tag, name=tag, bufs=bufs)

        for b in range(kt // K):
            c = wt([128, K * 3], f32, "c")
            nc.sync.dma_start(c[:], crd3[:, b * K * 3:(b + 1) * K * 3])

            pix = wt([128, K * 3], f32, "pix")
            nc.scalar.activation(pix[:], c[:], AF.Copy, bias=255.5, scale=255.5)
            # HW f32->i32 cast is rint; cast(pix - 0.5) == floor(pix) for
            # non-integer pix (integer pix may give pix-1 with fr=1.0, which is
            # bilinear-equivalent).
            pixm = wt([128, K * 3], f32, "pixm")
            nc.scalar.activation(pixm[:], c[:], AF.Copy, bias=255.0, scale=255.5)
            ci = wt([128, K * 3], i32, "ci")
            nc.gpsimd.tensor_copy(ci[:], pixm[:])
            cf = wt([128, K * 3], f32, "cf")
            nc.gpsimd.tensor_copy(cf[:], ci[:])
            fr = wt([128, K * 3], f32, "fr")
            nc.vector.tensor_sub(fr[:], pix[:], cf[:])
            omf = wt([128, K * 3], f32, "omf")
            nc.scalar.activation(omf[:], fr[:], AF.Copy, bias=1.0, scale=-1.0)

            ci3 = ci[:].rearrange("p (k c) -> p k c", c=3)
            fr3 = fr[:].rearrange("p (k c) -> p k c", c=3)
            omf3 = omf[:].rearrange("p (k c) -> p k c", c=3)

            idx = wt([128, 3 * K], i32, "idx")
            idx3 = idx[:].rearrange("p (pl k) -> p pl k", pl=3)
            for pl, (xc, yc) in enumerate(PAIRS):
                if pl == 0:
                    nc.vector.scalar_tensor_tensor(
                        out=idx3[:, 0], in0=ci3[:, :, yc], scalar=RES,
                        in1=ci3[:, :, xc], op0=mult, op1=add)
                else:
                    t1 = wt([128, K], i32, "t1", bufs=3)
                    nc.vector.scalar_tensor_tensor(
                        out=t1[:], in0=ci3[:, :, yc], scalar=RES,
                        in1=ci3[:, :, xc], op0=mult, op1=add)
                    nc.vector.tensor_scalar_add(idx3[:, pl], t1[:], pl * CELLS)

            # corner weights: [(1-fy)(1-fx), (1-fy)fx, fy(1-fx), fy fx]
            wts = wt([128, 3 * 4 * K], f32, "wts")
            wts4 = wts[:].rearrange("p (pl c k) -> p pl c k", pl=3, c=4)
            for pl, (xc, yc) in enumerate(PAIRS):
                fx, fy = fr3[:, :, xc], fr3[:, :, yc]
                gx, gy = omf3[:, :, xc], omf3[:, :, yc]
                eng = nc.vector
                eng.tensor_tensor(out=wts4[:, pl, 0], in0=gy, in1=gx, op=mult)
                eng.tensor_tensor(out=wts4[:, pl, 1], in0=gy, in1=fx, op=mult)
                eng.tensor_tensor(out=wts4[:, pl, 2], in0=fy, in1=gx, op=mult)
                eng.tensor_tensor(out=wts4[:, pl, 3], in0=fy, in1=fx, op=mult)

            # HW indirect DMA uses ONE offset per partition and fetches the
            # whole dest row contiguously, so each (plane, point) patch fetch
            # needs its own [P,1]-offset gather.
            g = gpool.tile([128, 3 * K * 128], tdt, tag="g", name="g")
            for col in range(3 * K):
                nc.gpsimd.indirect_dma_start(
                    out=g[:, col * 128:(col + 1) * 128], out_offset=None,
                    in_=ptd[:],
                    in_offset=_ioa()(ap=idx[:, col:col + 1], axis=0))
            g5 = g[:].rearrange("p (pl k c f) -> p pl k c f", pl=3, c=4, f=EMB)

            # all combine on DVE: Pool stays free for SWDGE desc-gen
            acc = wt([128, K * EMB], f32, "accA")
            acc3 = acc[:].rearrange("p (k f) -> p k f", f=EMB)
            terms = [(pl, cc) for pl in range(3) for cc in range(4)]
            feats = wt([128, K * EMB], bf16, "feats")
            for i, (pl, cc) in enumerate(terms):
                w_b = wts4[:, pl, cc].unsqueeze(2).to_broadcast([128, K, EMB])
                gsl = g5[:, pl, :, cc]
                if i == 0:
                    nc.vector.tensor_tensor(out=acc3, in0=gsl, in1=w_b, op=mult)
                    continue
                prod = wt([128, K * EMB], f32, "prodA", bufs=2)
                nc.vector.tensor_tensor(
                    out=prod[:].rearrange("p (k f) -> p k f", f=EMB),
                    in0=gsl, in1=w_b, op=mult)
                if i == len(terms) - 1:
                    nc.vector.tensor_add(out=feats[:], in0=acc[:], in1=prod[:])
                else:
                    nc.vector.tensor_add(out=acc[:], in0=acc[:], in1=prod[:])

            # MLP
            for j in range(NBATCH):
                k0 = b * K + j * BATCH
                ftp = psum.tile([EMB, BATCH * 128], bf16, tag="ftp", name="ftp",
                                space="PSUM", bufs=2)
                for kk in range(BATCH):
                    nc.tensor.transpose(
                        out=ftp[:, kk * 128:(kk + 1) * 128],
                        in_=feats[:, (j * BATCH + kk) * EMB:(j * BATCH + kk + 1) * EMB],
                        identity=ident[:])
                fts = wt([EMB, BATCH * 128], bf16, "fts")
                nc.scalar.activation(fts[:], ftp[:], AF.Copy)

                mm0 = psum.tile([HID, BATCH * 128], f32, tag="mm", name="mm",
                                space="PSUM", bufs=3)
                nc.tensor.matmul(out=mm0[:], lhsT=w0s[:], rhs=fts[:],
                                 start=True, stop=True)
                h0 = wt([HID, BATCH * 128], bf16, "h0")
                nc.scalar.activation(h0[:], mm0[:], AF.Relu, bias=b0s[:, 0:1])

                mm1 = psum.tile([HID, BATCH * 128], f32, tag="mm", name="mm",
                                space="PSUM", bufs=3)
                nc.tensor.matmul(out=mm1[:], lhsT=w1s[:], rhs=h0[:],
                                 start=True, stop=True)
                h1 = wt([HID, BATCH * 128], bf16, "h1")
                nc.scalar.activation(h1[:], mm1[:], AF.Relu, bias=b1s[:, 0:1])

                mm2 = psum.tile([HID, BATCH * 128], f32, tag="mm", name="mm",
                                space="PSUM", bufs=3)
                nc.tensor.matmul(out=mm2[:], lhsT=w2s[:], rhs=h1[:],
                                 start=True, stop=True)
                h2 = wt([HID, BATCH * 128], bf16, "h2")
                nc.scalar.activation(h2[:], mm2[:], AF.Relu, bias=b2s[:, 0:1])

                mm3 = psum.tile([1, BATCH * 128], f32, tag="mm3", name="mm3",
                                space="PSUM", bufs=2)
                nc.tensor.matmul(out=mm3[:], lhsT=w3s[:], rhs=h2[:],
                                 start=True, stop=True)
                res = wt([1, BATCH * 128], f32, "res")
                nc.scalar.activation(res[:], mm3[:], AF.Identity,
                                     bias=b3s[0:1, 0:1])
                nc.sync.dma_start(outv[:, k0 * 128:(k0 + BATCH) * 128], res[:])

    if do_finalize:
        nc.finalize()
    return nc


def _ioa():
    from concourse import bass
    return bass.IndirectOffsetOnAxis


def _get_nc():
    key = "float32" if TABLE_F32 else "bfloat16"
    if key not in _BUILT:
        _BUILT[key] = _build_nc(key)
    return _BUILT[key]


def _build_patch_table(planes: np.ndarray, np_dt) -> np.ndarray:
    # planes [3, 32, 512, 512] -> PT [3*512*512, 128]
    p = planes.transpose(0, 2, 3, 1)  # [3, H, W, C]
    pt = np.zeros((3, RES, RES, 4, EMB), dtype=np.float32)
    pt[:, :, :, 0] = p
    pt[:, :, :-1, 1] = p[:, :, 1:]
    pt[:, :-1, :, 2] = p[:, 1:]
    pt[:, :-1, :-1, 3] = p[:, 1:, 1:]
    return np.ascontiguousarray(pt.reshape(3 * CELLS, 4 * EMB)).astype(np_dt)


def kernel(**inputs: np.ndarray) -> np.ndarray:
    global LAST_RESULTS
    import ml_dtypes
    from concourse.bass_utils import run_bass_kernel_spmd

    coords = np.asarray(inputs["coordinates"], dtype=np.float32)
    planes = np.asarray(inputs["planes"], dtype=np.float32)
    bf = ml_dtypes.bfloat16
    np_tdt = np.float32 if TABLE_F32 else bf
    pt = _build_patch_table(planes, np_tdt)
    w0t = np.ascontiguousarray(inputs["w0"].T).astype(bf)
    w1t = np.ascontiguousarray(inputs["w1"].T).astype(bf)
    w2t = np.ascontiguousarray(inputs["w2"].T).astype(bf)
    w3t = np.ascontiguousarray(inputs["w3"].T).astype(bf)
    b0 = np.asarray(inputs["b0"], np.float32).reshape(HID, 1)
    b1 = np.asarray(inputs["b1"], np.float32).reshape(HID, 1)
    b2 = np.asarray(inputs["b2"], np.float32).reshape(HID, 1)
    b3 = np.asarray(inputs["b3"], np.float32).reshape(1, 1)

    n = coords.shape[0]
    coords_pad = np.zeros((NCORES * NP, 3), np.float32)
    coords_pad[:n] = coords

    in_maps = []
    for i in range(NCORES):
        in_maps.append({
            "pt": pt,
            "coords": np.ascontiguousarray(coords_pad[i * NP:(i + 1) * NP]),
            "w0t": w0t, "w1t": w1t, "w2t": w2t, "w3t": w3t,
            "b0c": b0, "b1c": b1, "b2c": b2, "b3c": b3,
        })

    nc = _get_nc()
    LAST_RESULTS = run_bass_kernel_spmd(nc, in_maps, list(range(NCORES)))
    pieces = []
    for i in range(NCORES):
        o = np.asarray(LAST_RESULTS.results[i]["out"], np.float32)
        pieces.append(o.reshape(KT, 128).T.ravel())  # -> point order p*KT+k
    full = np.concatenate(pieces)[:n]
    return full.reshape(1, n, 1).astype(np.float32)
